# revision 19
# baseline (speedup 1.0000x reference)
"""Trainium2 Bass kernel for nn_ContrastiveLossV2 (8-core SPMD), v2.

Reference computation:
    z = l2norm(concat([emb_i, emb_j]))          # [8192, 128]
    sim = z @ z.T                               # [8192, 8192]
    loss = mean((sim - class_pairs)**2)

Expanded square (no sim materialization):
    sum((sim-cp)^2) = sum(sim^2) - 2*sum(sim*cp) + sum(cp^2)
  * sum(sim^2)  = ||Z^T Z||_F^2 via the local Gram (host-combined in f64)
  * sum(sim*cp) = sum_{d,c} V[d,c] * zT[d,c],  V = Z_loc^T @ CP_loc
  * sum(cp^2)   = square+accumulate passes split across Act / DVE / Pool /
                  TensorE (X^T X diag-accumulate trick)

v3 layout/engine changes vs the f32-streaming v1 baseline (104.8us):
  * class_pairs is cast to fp8 e4m3 on the host (8MB/core vs 32MB f32);
    emb is staged twice: row-block-major bf16 [128 r, 64 blk, 128 d]
    (d innermost -> DVE-packed norm pipeline) and d-major f8e4m3
    embt [128 d, 8192 c] for the zT side.
  * V = Z0^T CP runs as fp8 DoubleRow matmuls: block pairs contract 256
    rows per MM (64 MMs instead of 128), with the pair weights reused
    across PSUM-bank chunk sets so LDWEIGHTS count drops to 16.
  * zT = embt * w[col] runs on the otherwise-idle gpsimd engine; w is
    flattened to [1, 8192] via one PE transpose + DRAM bounce + a
    broadcast DMA read, killing v1's 64 PE transposes + PSUM copies.
  * cp^2 is split Act (0.92 ns/el-row Square+accum, the bulk) / DVE
    (fp8 STT+accum) / TensorE windows (X^T X accumulated into one PSUM
    bank all kernel long; only the diagonal, extracted once with an
    identity-mask dot, is meaningful = per-column sum of squares).
  * The gpsimd XYZWC cross-lane reduces (5.7us/instr in v1) are gone.
  * Input DMAs ride two HWDGE queues (SP: cp stream; ACT: emb/embt),
    with the emb bulk held behind cp group 0 so the squares/V pipeline
    starts as early as possible.
"""

import numpy as np

import concourse.bacc as bacc
import concourse.mybir as mybir
import concourse.tile as tile
from concourse.bass_utils import run_bass_kernel_spmd
from concourse.tile_rust import add_dep_helper

f32 = mybir.dt.float32
bf16 = mybir.dt.bfloat16
f8e3 = mybir.dt.float8e4   # e4m3: class_pairs (DoubleRow needs e4/e5)
f8e4 = mybir.dt.float8e4   # e4m3: embt staging
AF = mybir.ActivationFunctionType
OP = mybir.AluOpType

N_CORES = 8
N, D = 4096, 128
TWO_N = 2 * N                      # 8192
R_LOC = TWO_N // N_CORES           # 1024 rows per core
M_BLK = R_LOC // 128               # 8 local 128-row blocks
NB = TWO_N // 128                  # 64 total row blocks
NCH = 512                          # matmul free-dim chunk (one PSUM bank)
GROUP_W = [1024] * 7 + [512, 512]  # cp column-group widths (sum = 8192)
N_GRP = len(GROUP_W)
NPIECE = [(0, 8), (8, 32), (32, 64)]   # norm pieces (block ranges)
WPIECE = [(0, 32), (32, 64)]           # w-flatten / zT pieces
EPS = 1e-12

# cp^2 split per group: [0,a) Act square, [a,a+p) Pool two-pass,
# [a+p,a+p+d) DVE STT, rest TensorE windows (multiple of 128).
# Rates (ns/el-row, measured): Act 0.91, Pool ~2+3.9 (TT + XYZWC), DVE f8
# ~2.7, TensorE ~0.65 (LDW+MM N=128 warm).
ACT_F, POOL_F, DVE_F = 0.70, 0.0, 0.14
SPLITS = {}
for _gw in (1024, 512):
    _tot = M_BLK * _gw
    _a = (int(_tot * ACT_F) // 16) * 16
    _p = (int(_tot * POOL_F) // 16) * 16
    _d = (int(_tot * DVE_F) // 16) * 16
    _t = _tot - _a - _p - _d
    _t -= _t % 128
    _d = _tot - _a - _p - _t
    SPLITS[_gw] = (_a, _p, _d, _t)

# accumulator columns: act | pool | dve | cross (per 512-chunk) | diag
N_CHK = TWO_N // NCH               # 16 cross chunks
A_COL, P_COL, D_COL = 0, N_GRP, 2 * N_GRP
X_COL = 3 * N_GRP
Q_COL = X_COL + N_CHK
ACC_W = Q_COL + 1

_cached = {}


def _offs(widths):
    offs, o = [], 0
    for w in widths:
        offs.append(o)
        o += w
    return offs


GRP_OFF = _offs(GROUP_W)
N_SQW = sum(SPLITS[gw][3] // 128 for gw in GROUP_W)  # total tensor windows


def _build_module():
    nc = bacc.Bacc("TRN2", target_bir_lowering=False, debug=False,
                   num_devices=N_CORES)

    # staged inputs (host pre-arranged, see kernel()):
    #  embs: [128 r, 64 blk, 128 d] bf16, rotated so blocks 0..7 are local
    #  embt: [128 d, 8192 c] f8e4m3, same rotated column order
    #  cpst: [128, 65536] f8e3m4, rotated rows+cols, group-tiled (as v1)
    embs = nc.dram_tensor("embs", [128, NB, D], bf16, kind="ExternalInput")
    embt = nc.dram_tensor("embt", [128, TWO_N], f8e4, kind="ExternalInput")
    cpst = nc.dram_tensor("cpst", [128, M_BLK * TWO_N], f8e3,
                          kind="ExternalInput")
    ident = nc.dram_tensor("ident", [128, 128], bf16, kind="ExternalInput")
    out_g = nc.dram_tensor("out_g", [128, 128], f32, kind="ExternalOutput")
    out_acc = nc.dram_tensor("out_acc", [128, ACC_W], f32,
                             kind="ExternalOutput")
    scr = [nc.dram_tensor(f"scr{p}", [32, 128], bf16, kind="Internal")
           for p in range(len(WPIECE))]

    with tile.TileContext(nc) as tc:
        with (
            tc.tile_pool(name="const", bufs=1) as const_pool,
            tc.tile_pool(name="persist", bufs=1) as persist,
            tc.tile_pool(name="sq", bufs=2) as sq_pool,
            tc.tile_pool(name="norm", bufs=3) as norm_pool,
            tc.tile_pool(name="wt", bufs=2) as wt_pool,
            tc.tile_pool(name="wb", bufs=2) as wb_pool,
            tc.tile_pool(name="cpt", bufs=9) as cp_pool,
            tc.tile_pool(name="tmp", bufs=4) as tmp_pool,
            tc.tile_pool(name="sqa", bufs=2) as sqa_pool,
            tc.tile_pool(name="sqp", bufs=2) as sqp_pool,
            tc.tile_pool(name="sqd", bufs=2) as sqd_pool,
            tc.tile_pool(name="psv", bufs=5, space="PSUM") as psv_pool,
            tc.tile_pool(name="psq", bufs=1, space="PSUM") as psq_pool,
            tc.tile_pool(name="psx", bufs=1, space="PSUM") as psx_pool,
        ):
            ident_sb = const_pool.tile([128, 128], bf16)
            stag = persist.tile([128, NB, D], bf16)
            embt_sb = persist.tile([128, TWO_N], f8e4)
            zt = persist.tile([128, TWO_N], bf16)
            z0 = persist.tile([128, M_BLK, D], f8e3)
            w_all = persist.tile([128, NB], f32)
            acc = persist.tile([128, ACC_W], f32)

            # ---- input DMAs on two HWDGE queues
            sp_dmas = []
            cpts = []
            for g, gw in enumerate(GROUP_W):
                cpt = cp_pool.tile([128, M_BLK * gw], f8e3, tag="cpt",
                                   name="cpt",
                                   padded_shape=[128, M_BLK * GROUP_W[0]])
                off = M_BLK * GRP_OFF[g]
                sp_dmas.append(nc.sync.dma_start(
                    out=cpt[:], in_=cpst[:, off:off + M_BLK * gw]))
                cpts.append(cpt)
            for a, b in zip(sp_dmas, sp_dmas[1:]):
                add_dep_helper(b.ins, a.ins, False, "SP DMA queue order")

            act_dmas = [
                nc.scalar.dma_start(out=ident_sb[:], in_=ident[:]),
                nc.scalar.dma_start(out=stag[:, 0:8, :],
                                    in_=embs[:, 0:8, :]),
                nc.scalar.dma_start(out=stag[:, 8:32, :],
                                    in_=embs[:, 8:32, :]),
                nc.scalar.dma_start(out=stag[:, 32:64, :],
                                    in_=embs[:, 32:64, :]),
                nc.scalar.dma_start(out=embt_sb[:], in_=embt[:]),
            ]
            for a, b in zip(act_dmas, act_dmas[1:]):
                add_dep_helper(b.ins, a.ins, False, "ACT DMA queue order")
            # hold the 2.9MB emb-staging tail until cp group 0 has landed
            # so the squares/V pipeline starts ~4us earlier
            add_dep_helper(act_dmas[2].ins, sp_dmas[0].ins, False,
                           "cp g0 priority over emb bulk")

            # ---- norm pipeline (per piece): nsq -> sqrt -> max -> 1/x
            def do_norm(p):
                b0, b1 = NPIECE[p]
                nb = b1 - b0
                sqt = sq_pool.tile([128, nb, D], bf16, tag="sqt", name="sqt",
                                   padded_shape=[128, 32, D])
                nc.vector.tensor_tensor(sqt[:], stag[:, b0:b1, :],
                                        stag[:, b0:b1, :], op=OP.mult)
                nsq = norm_pool.tile([128, nb], f32, tag="nsq", name="nsq",
                                     padded_shape=[128, 32])
                nc.vector.tensor_reduce(nsq[:], sqt[:],
                                        axis=mybir.AxisListType.X, op=OP.add)
                nrm = norm_pool.tile([128, nb], f32, tag="nrm", name="nrm",
                                     padded_shape=[128, 32])
                nc.scalar.activation(nrm[:], nsq[:], AF.Sqrt)
                nc.vector.tensor_scalar_max(nrm[:], nrm[:], EPS)
                nc.vector.reciprocal(w_all[:, b0:b1], nrm[:])

            def do_z0():
                with nc.allow_low_precision(
                        reason="z0 in fp8e4m3 for DoubleRow matmul; "
                               "validated 2.6e-4 loss err vs 2e-2 gate"):
                    for b in range(M_BLK):
                        nc.vector.tensor_scalar_mul(
                            z0[:, b, :], stag[:, b, :], w_all[:, b:b + 1])

            # ---- w flatten chain per wpiece: PE transpose -> copy ->
            # DRAM bounce -> [1, 4096] -> partition_broadcast
            def do_wchain(p):
                w0, w1 = WPIECE[p]
                w_bf = wt_pool.tile([128, w1 - w0], bf16, tag="wbf",
                                    name="wbf")
                with nc.allow_low_precision(
                        reason="w in bf16 for the PE transpose: ~0.4% w "
                               "err is ~1e-4 on the loss, gate is 2e-2"):
                    nc.vector.tensor_copy(w_bf[:], w_all[:, w0:w1])
                wt_ps = psx_pool.tile([32, 128], bf16, tag="wtps",
                                      name="wtps")
                nc.tensor.transpose(wt_ps[:], w_bf[:], ident_sb[:])
                wt_sb = wt_pool.tile([32, 128], bf16, tag="wtsb", name="wtsb")
                nc.vector.tensor_copy(wt_sb[:], wt_ps[:])
                nc.scalar.dma_start(out=scr[p][:], in_=wt_sb[:])

            def do_pb(p):
                w0, w1 = WPIECE[p]
                wB = wb_pool.tile([128, (w1 - w0) * 128], bf16, tag="wB",
                                  name="wB")
                nc.scalar.dma_start(
                    out=wB[:],
                    in_=scr[p][:].rearrange("b r -> () (b r)")
                    .broadcast_to([128, (w1 - w0) * 128]))
                return wB

            def do_zt(g, wB, p):
                w0, _ = WPIECE[p]
                col0 = GRP_OFF[g]
                gw = GROUP_W[g]
                lo = col0 - w0 * 128
                nc.gpsimd.tensor_tensor(zt[:, col0:col0 + gw],
                                        embt_sb[:, col0:col0 + gw],
                                        wB[:, lo:lo + gw], op=OP.mult)

            def do_gram():
                g_ps = psx_pool.tile([128, 128], f32, tag="gram", name="gram")
                for m in range(M_BLK):
                    nc.tensor.matmul(g_ps[:], lhsT=z0[:, m, :],
                                     rhs=z0[:, m, :],
                                     start=(m == 0), stop=(m == M_BLK - 1))
                g_sb = tmp_pool.tile([128, 128], f32, tag="gsb", name="gsb")
                nc.scalar.copy(g_sb[:], g_ps[:])
                return g_sb

            # ---- squares: tensor windows accumulate into one PSUM bank
            sq_ps = psq_pool.tile([128, 128], f32)
            sqw_done = [0]

            def do_sq_windows(g):
                gw = GROUP_W[g]
                tot = M_BLK * gw
                a, p, d, t = SPLITS[gw]
                cpt = cpts[g]
                for c in range(a + p + d, tot, 128):
                    first = sqw_done[0] == 0
                    sqw_done[0] += 1
                    last = sqw_done[0] == N_SQW
                    nc.tensor.matmul(sq_ps[:], lhsT=cpt[:, c:c + 128],
                                     rhs=cpt[:, c:c + 128],
                                     start=first, stop=last)

            def _chunk_group(i):
                c0 = i * NCH
                for g, (off, gw) in enumerate(zip(GRP_OFF, GROUP_W)):
                    if off <= c0 < off + gw:
                        return g, c0 - off
                raise AssertionError(i)

            def do_v_set(i0, n):
                # V = Z0^T @ CP in fp8 DoubleRow pairs: each matmul
                # contracts 256 rows (two 128-row blocks).  The t-loop is
                # OUTER so the pair weights are reused across the n chunks
                # of the set (each chunk accumulates in its own PSUM bank).
                pss = [psv_pool.tile([128, NCH], f32, tag="psv", name="psv")
                       for _ in range(n)]
                for t in range(M_BLK // 2):
                    for j in range(n):
                        g, koff = _chunk_group(i0 + j)
                        cpv = cpts[g][:].rearrange("q (m w) -> q m w",
                                                   m=M_BLK)
                        nc.tensor.matmul(
                            pss[j][:], lhsT=z0[:, 2 * t:2 * t + 2, :],
                            rhs=cpv[:, 2 * t:2 * t + 2, koff:koff + NCH],
                            start=(t == 0), stop=(t == M_BLK // 2 - 1),
                            perf_mode=mybir.MatmulPerfMode.DoubleRow)
                return pss

            def do_sja(g):
                gw = GROUP_W[g]
                a, _, _, _ = SPLITS[gw]
                cpt = cpts[g]
                pad_a = SPLITS[GROUP_W[0]][0]
                sja = sqa_pool.tile([128, a], f8e3, tag="sja", name="sja",
                                    padded_shape=[128, pad_a])
                nc.scalar.activation(sja[:], cpt[:, 0:a], AF.Square,
                                     accum_out=acc[:, A_COL + g:A_COL + g + 1])

            def do_sjp(g):
                gw = GROUP_W[g]
                a, p, _, _ = SPLITS[gw]
                if p == 0:
                    return
                cpt = cpts[g]
                pad_p = SPLITS[GROUP_W[0]][1]
                sjp = sqp_pool.tile([128, p], bf16, tag="sjp", name="sjp",
                                    padded_shape=[128, pad_p])
                nc.gpsimd.tensor_tensor(sjp[:], cpt[:, a:a + p],
                                        cpt[:, a:a + p], op=OP.mult)
                nc.gpsimd.tensor_reduce(acc[0:1, P_COL + g:P_COL + g + 1],
                                        sjp[:], axis=mybir.AxisListType.XYZWC,
                                        op=OP.add)

            def do_sjd(g):
                gw = GROUP_W[g]
                a, p, d, _ = SPLITS[gw]
                cpt = cpts[g]
                pad_d = SPLITS[GROUP_W[0]][2]
                sjd = sqd_pool.tile([128, d], f8e3, tag="sjd", name="sjd",
                                    padded_shape=[128, pad_d])
                nc.vector.scalar_tensor_tensor(
                    out=sjd[:], in0=cpt[:, a + p:a + p + d], scalar=1.0,
                    in1=cpt[:, a + p:a + p + d], op0=OP.mult, op1=OP.mult,
                    accum_out=acc[:, D_COL + g:D_COL + g + 1])

            def do_xj(ci, ps):
                xj = tmp_pool.tile([128, NCH], bf16, tag="xj", name="xj")
                nc.vector.scalar_tensor_tensor(
                    out=xj[:], in0=ps[:], scalar=1.0,
                    in1=zt[:, ci * NCH:(ci + 1) * NCH],
                    op0=OP.mult, op1=OP.mult,
                    accum_out=acc[:, X_COL + ci:X_COL + ci + 1])

            def do_diag():
                dtmp = tmp_pool.tile([128, 128], bf16, tag="dtmp",
                                     name="dtmp")
                nc.vector.scalar_tensor_tensor(
                    out=dtmp[:], in0=sq_ps[:], scalar=1.0, in1=ident_sb[:],
                    op0=OP.mult, op1=OP.mult,
                    accum_out=acc[:, Q_COL:Q_COL + 1])

            # ---- issue schedule (engine queues are FIFO: order matters)
            def sq_group(g):
                do_sq_windows(g)
                do_sja(g)
                do_sjp(g)
                do_sjd(g)

            do_norm(0)
            do_z0()
            sq_group(0)
            do_norm(1)
            sq_group(1)
            do_norm(2)
            do_wchain(0)
            g_sb = do_gram()
            wB0 = do_pb(0)
            sq_group(2)
            set0 = do_v_set(0, 5)        # chunks 0-4 (groups 0-2)
            do_zt(0, wB0, 0)
            do_xj(0, set0[0])
            do_xj(1, set0[1])
            sq_group(3)
            do_zt(1, wB0, 0)
            do_xj(2, set0[2])
            do_xj(3, set0[3])
            sq_group(4)
            do_wchain(1)
            wB1 = do_pb(1)
            do_zt(2, wB0, 0)
            do_xj(4, set0[4])
            set1 = do_v_set(5, 5)        # chunks 5-9 (groups 2-4)
            do_zt(3, wB0, 0)
            do_xj(5, set1[0])
            do_xj(6, set1[1])
            do_xj(7, set1[2])
            sq_group(5)
            do_zt(4, wB1, 1)
            do_xj(8, set1[3])
            do_xj(9, set1[4])
            sq_group(6)
            set2 = do_v_set(10, 4)       # chunks 10-13 (groups 5-6)
            do_zt(5, wB1, 1)
            do_xj(10, set2[0])
            do_xj(11, set2[1])
            sq_group(7)
            do_zt(6, wB1, 1)
            do_xj(12, set2[2])
            do_xj(13, set2[3])
            sq_group(8)
            set3 = do_v_set(14, 2)       # chunks 14-15 (groups 7-8)
            do_zt(7, wB1, 1)
            do_xj(14, set3[0])
            do_zt(8, wB1, 1)
            do_xj(15, set3[1])
            do_diag()

            nc.scalar.dma_start(out=out_g[:], in_=g_sb[:])
            nc.scalar.dma_start(out=out_acc[:], in_=acc[:])

    nc.compile()
    return nc


def _get_module():
    if "nc" not in _cached:
        _cached["nc"] = _build_module()
    return _cached["nc"]


def kernel(emb_i, emb_j, class_pairs, _return_raw=False, _trace=False):
    import ml_dtypes

    emb_i = np.ascontiguousarray(emb_i, dtype=np.float32)
    emb_j = np.ascontiguousarray(emb_j, dtype=np.float32)
    class_pairs = np.ascontiguousarray(class_pairs, dtype=np.float32)
    ident = np.eye(128, dtype=ml_dtypes.bfloat16)

    emb = np.concatenate([emb_i, emb_j], axis=0)          # [8192, 128]
    emb_blocks = emb.reshape(NB, 128, D)
    cp_f8 = class_pairs.astype(ml_dtypes.float8_e4m3)     # host-side cast

    nc = _get_module()
    in_maps = []
    for c in range(N_CORES):
        order = [(M_BLK * c + k) % NB for k in range(NB)]
        rb = emb_blocks[order]                            # [64, 128, 128]
        embs_st = np.ascontiguousarray(
            rb.transpose(1, 0, 2)).astype(ml_dtypes.bfloat16)
        embt_st = np.ascontiguousarray(
            rb.transpose(2, 0, 1).reshape(D, TWO_N)).astype(
                ml_dtypes.float8_e4m3)

        r0 = c * R_LOC
        rows = cp_f8[r0:r0 + R_LOC]                       # [1024, 8192]
        s = r0 % TWO_N
        rot = np.concatenate([rows[:, s:], rows[:, :s]], axis=1)
        rb8 = rot.reshape(M_BLK, 128, TWO_N)              # [8, 128, 8192]
        parts = []
        for gw, off in zip(GROUP_W, GRP_OFF):
            parts.append(rb8[:, :, off:off + gw].transpose(1, 0, 2)
                         .reshape(128, M_BLK * gw))
        cp_st = np.ascontiguousarray(np.concatenate(parts, axis=1))

        in_maps.append({"embs": embs_st, "embt": embt_st, "cpst": cp_st,
                        "ident": ident})

    res = run_bass_kernel_spmd(nc, in_maps, list(range(N_CORES)),
                               trace=_trace)

    G = np.zeros((128, 128), dtype=np.float64)
    sum_cp2 = 0.0
    cross = 0.0
    for c in range(N_CORES):
        G += res.results[c]["out_g"].astype(np.float64)
        out = res.results[c]["out_acc"].astype(np.float64)
        sum_cp2 += out[:, A_COL:P_COL].sum()    # act partials
        if POOL_F > 0:
            sum_cp2 += out[0, P_COL:D_COL].sum()  # pool scalars (row 0)
        sum_cp2 += out[:, D_COL:X_COL].sum()    # dve partials
        sum_cp2 += out[:, Q_COL].sum()          # tensor diag column
        cross += out[:, X_COL:Q_COL].sum()
    sum_sim2 = float((G * G).sum())
    loss = (sum_sim2 - 2.0 * cross + sum_cp2) / float(TWO_N * TWO_N)
    out = np.asarray(loss, dtype=np.float32)
    if _return_raw:
        return out, res
    return out


# revision 20
# speedup vs baseline: 1.0283x; 1.0283x over previous
"""Trainium2 Bass kernel for nn_ContrastiveLossV2 (8-core SPMD), v2.

Reference computation:
    z = l2norm(concat([emb_i, emb_j]))          # [8192, 128]
    sim = z @ z.T                               # [8192, 8192]
    loss = mean((sim - class_pairs)**2)

Expanded square (no sim materialization):
    sum((sim-cp)^2) = sum(sim^2) - 2*sum(sim*cp) + sum(cp^2)
  * sum(sim^2)  = ||Z^T Z||_F^2 via the local Gram (host-combined in f64)
  * sum(sim*cp) = sum_{d,c} V[d,c] * zT[d,c],  V = Z_loc^T @ CP_loc
  * sum(cp^2)   = square+accumulate passes split across Act / DVE / Pool /
                  TensorE (X^T X diag-accumulate trick)

v3 layout/engine changes vs the f32-streaming v1 baseline (104.8us):
  * class_pairs is cast to fp8 e4m3 on the host (8MB/core vs 32MB f32);
    emb is staged twice: row-block-major bf16 [128 r, 64 blk, 128 d]
    (d innermost -> DVE-packed norm pipeline) and d-major f8e4m3
    embt [128 d, 8192 c] for the zT side.
  * V = Z0^T CP runs as fp8 DoubleRow matmuls: block pairs contract 256
    rows per MM (64 MMs instead of 128), with the pair weights reused
    across PSUM-bank chunk sets so LDWEIGHTS count drops to 16.
  * zT = embt * w[col] runs on the otherwise-idle gpsimd engine; w is
    flattened to [1, 8192] via one PE transpose + DRAM bounce + a
    broadcast DMA read, killing v1's 64 PE transposes + PSUM copies.
  * cp^2 is split Act (0.92 ns/el-row Square+accum, the bulk) / DVE
    (fp8 STT+accum) / TensorE windows (X^T X accumulated into one PSUM
    bank all kernel long; only the diagonal, extracted once with an
    identity-mask dot, is meaningful = per-column sum of squares).
  * The gpsimd XYZWC cross-lane reduces (5.7us/instr in v1) are gone.
  * Input DMAs ride two HWDGE queues (SP: cp stream; ACT: emb/embt),
    with the emb bulk held behind cp group 0 so the squares/V pipeline
    starts as early as possible.
"""

import numpy as np

import concourse.bacc as bacc
import concourse.mybir as mybir
import concourse.tile as tile
from concourse.bass_utils import run_bass_kernel_spmd
from concourse.tile_rust import add_dep_helper

f32 = mybir.dt.float32
bf16 = mybir.dt.bfloat16
f8e3 = mybir.dt.float8e4   # e4m3: class_pairs (DoubleRow needs e4/e5)
f8e4 = mybir.dt.float8e4   # e4m3: embt staging
AF = mybir.ActivationFunctionType
OP = mybir.AluOpType

N_CORES = 8
N, D = 4096, 128
TWO_N = 2 * N                      # 8192
R_LOC = TWO_N // N_CORES           # 1024 rows per core
M_BLK = R_LOC // 128               # 8 local 128-row blocks
NB = TWO_N // 128                  # 64 total row blocks
NCH = 512                          # matmul free-dim chunk (one PSUM bank)
GROUP_W = [1024] * 7 + [512, 512]  # cp column-group widths (sum = 8192)
N_GRP = len(GROUP_W)
NPIECE = [(0, 8), (8, 32), (32, 64)]   # norm pieces (block ranges)
WPIECE = [(0, 32), (32, 64)]           # w-flatten / zT pieces
EPS = 1e-12

# cp^2 split per group: [0,a) Act square, [a,a+p) Pool two-pass,
# [a+p,a+p+d) DVE STT, rest TensorE windows (multiple of 128).
# Rates (ns/el-row, measured): Act 0.91, Pool ~2+3.9 (TT + XYZWC), DVE f8
# ~2.7, TensorE ~0.65 (LDW+MM N=128 warm).
ACT_F, POOL_F, DVE_F = 0.62, 0.0, 0.14
SPLITS = {}
for _gw in (1024, 512):
    _tot = M_BLK * _gw
    _a = (int(_tot * ACT_F) // 16) * 16
    _p = (int(_tot * POOL_F) // 16) * 16
    _d = (int(_tot * DVE_F) // 16) * 16
    _t = _tot - _a - _p - _d
    _t -= _t % 128
    _d = _tot - _a - _p - _t
    SPLITS[_gw] = (_a, _p, _d, _t)

# accumulators are split BY WRITER ENGINE so accum_out writes never
# create cross-engine tile write-order chains:
#   acc_a [128, N_GRP]: Act square partials
#   acc_v [128, N_GRP + N_CHK + 1]: DVE partials (sjd | cross | diag)
N_CHK = TWO_N // NCH               # 16 cross chunks
D_COL = 0
X_COL = N_GRP
Q_COL = X_COL + N_CHK
ACC_W = Q_COL + 1

_cached = {}


def _offs(widths):
    offs, o = [], 0
    for w in widths:
        offs.append(o)
        o += w
    return offs


GRP_OFF = _offs(GROUP_W)
N_SQW = sum(SPLITS[gw][3] // 128 for gw in GROUP_W)  # total tensor windows


def _build_module():
    nc = bacc.Bacc("TRN2", target_bir_lowering=False, debug=False,
                   num_devices=N_CORES)

    # staged inputs (host pre-arranged, see kernel()):
    #  embs: [128 r, 64 blk, 128 d] bf16, rotated so blocks 0..7 are local
    #  embt: [128 d, 8192 c] f8e4m3, same rotated column order
    #  cpst: [128, 65536] f8e3m4, rotated rows+cols, group-tiled (as v1)
    embs = nc.dram_tensor("embs", [128, NB, D], bf16, kind="ExternalInput")
    embt = nc.dram_tensor("embt", [128, TWO_N], f8e4, kind="ExternalInput")
    cpst = nc.dram_tensor("cpst", [128, M_BLK * TWO_N], f8e3,
                          kind="ExternalInput")
    ident = nc.dram_tensor("ident", [128, 128], bf16, kind="ExternalInput")
    out_g = nc.dram_tensor("out_g", [128, 128], f32, kind="ExternalOutput")
    out_acc_a = nc.dram_tensor("out_acc_a", [128, N_GRP], f32,
                               kind="ExternalOutput")
    out_acc = nc.dram_tensor("out_acc", [128, ACC_W], f32,
                             kind="ExternalOutput")
    scr = [nc.dram_tensor(f"scr{p}", [32, 128], bf16, kind="Internal")
           for p in range(len(WPIECE))]

    with tile.TileContext(nc) as tc:
        with (
            tc.tile_pool(name="const", bufs=1) as const_pool,
            tc.tile_pool(name="persist", bufs=1) as persist,
            tc.tile_pool(name="sq", bufs=2) as sq_pool,
            tc.tile_pool(name="norm", bufs=3) as norm_pool,
            tc.tile_pool(name="wt", bufs=2) as wt_pool,
            tc.tile_pool(name="wb", bufs=2) as wb_pool,
            tc.tile_pool(name="ztp", bufs=9) as zt_pool,
            tc.tile_pool(name="cpt", bufs=9) as cp_pool,
            tc.tile_pool(name="tmp", bufs=4) as tmp_pool,
            tc.tile_pool(name="sqa", bufs=2) as sqa_pool,
            tc.tile_pool(name="sqp", bufs=2) as sqp_pool,
            tc.tile_pool(name="sqd", bufs=2) as sqd_pool,
            tc.tile_pool(name="psv", bufs=5, space="PSUM") as psv_pool,
            tc.tile_pool(name="psq", bufs=1, space="PSUM") as psq_pool,
            tc.tile_pool(name="psx", bufs=1, space="PSUM") as psx_pool,
        ):
            ident_sb = const_pool.tile([128, 128], bf16)
            stag = persist.tile([128, NB, D], bf16)
            embt_sb = persist.tile([128, TWO_N], f8e4)
            z0 = persist.tile([128, M_BLK, D], f8e3)
            w_all = persist.tile([128, NB], f32)
            acc_a = persist.tile([128, N_GRP], f32)
            acc = persist.tile([128, ACC_W], f32)

            # ---- input DMAs on two HWDGE queues
            sp_dmas = []
            cpts = []
            for g, gw in enumerate(GROUP_W):
                cpt = cp_pool.tile([128, M_BLK * gw], f8e3, tag="cpt",
                                   name="cpt",
                                   padded_shape=[128, M_BLK * GROUP_W[0]])
                off = M_BLK * GRP_OFF[g]
                sp_dmas.append(nc.sync.dma_start(
                    out=cpt[:], in_=cpst[:, off:off + M_BLK * gw]))
                cpts.append(cpt)
            for a, b in zip(sp_dmas, sp_dmas[1:]):
                add_dep_helper(b.ins, a.ins, False, "SP DMA queue order")

            act_dmas = [
                nc.scalar.dma_start(out=ident_sb[:], in_=ident[:]),
                nc.scalar.dma_start(out=stag[:, 0:8, :],
                                    in_=embs[:, 0:8, :]),
                nc.scalar.dma_start(out=stag[:, 8:32, :],
                                    in_=embs[:, 8:32, :]),
                nc.scalar.dma_start(out=stag[:, 32:64, :],
                                    in_=embs[:, 32:64, :]),
                nc.scalar.dma_start(out=embt_sb[:], in_=embt[:]),
            ]
            for a, b in zip(act_dmas, act_dmas[1:]):
                add_dep_helper(b.ins, a.ins, False, "ACT DMA queue order")
            # hold the 2.9MB emb-staging tail until cp group 0 has landed
            # so the squares/V pipeline starts ~4us earlier
            add_dep_helper(act_dmas[2].ins, sp_dmas[0].ins, False,
                           "cp g0 priority over emb bulk")

            # ---- norm pipeline (per piece): nsq -> sqrt -> max -> 1/x
            def do_norm(p):
                b0, b1 = NPIECE[p]
                nb = b1 - b0
                sqt = sq_pool.tile([128, nb, D], bf16, tag="sqt", name="sqt",
                                   padded_shape=[128, 32, D])
                nc.vector.tensor_tensor(sqt[:], stag[:, b0:b1, :],
                                        stag[:, b0:b1, :], op=OP.mult)
                nsq = norm_pool.tile([128, nb], f32, tag="nsq", name="nsq",
                                     padded_shape=[128, 32])
                nc.vector.tensor_reduce(nsq[:], sqt[:],
                                        axis=mybir.AxisListType.X, op=OP.add)
                nrm = norm_pool.tile([128, nb], f32, tag="nrm", name="nrm",
                                     padded_shape=[128, 32])
                nc.scalar.activation(nrm[:], nsq[:], AF.Sqrt)
                nc.vector.tensor_scalar_max(nrm[:], nrm[:], EPS)
                nc.vector.reciprocal(w_all[:, b0:b1], nrm[:])

            def do_z0():
                with nc.allow_low_precision(
                        reason="z0 in fp8e4m3 for DoubleRow matmul; "
                               "validated 2.6e-4 loss err vs 2e-2 gate"):
                    for b in range(M_BLK):
                        nc.vector.tensor_scalar_mul(
                            z0[:, b, :], stag[:, b, :], w_all[:, b:b + 1])

            # ---- w flatten chain per wpiece: PE transpose -> copy ->
            # DRAM bounce -> [1, 4096] -> partition_broadcast
            def do_wchain(p):
                w0, w1 = WPIECE[p]
                w_bf = wt_pool.tile([128, w1 - w0], bf16, tag="wbf",
                                    name="wbf")
                with nc.allow_low_precision(
                        reason="w in bf16 for the PE transpose: ~0.4% w "
                               "err is ~1e-4 on the loss, gate is 2e-2"):
                    nc.vector.tensor_copy(w_bf[:], w_all[:, w0:w1])
                wt_ps = psx_pool.tile([32, 128], bf16, tag="wtps",
                                      name="wtps")
                nc.tensor.transpose(wt_ps[:], w_bf[:], ident_sb[:])
                wt_sb = wt_pool.tile([32, 128], bf16, tag="wtsb", name="wtsb")
                nc.vector.tensor_copy(wt_sb[:], wt_ps[:])
                nc.scalar.dma_start(out=scr[p][:], in_=wt_sb[:])

            def do_pb(p):
                w0, w1 = WPIECE[p]
                wB = wb_pool.tile([128, (w1 - w0) * 128], bf16, tag="wB",
                                  name="wB")
                nc.scalar.dma_start(
                    out=wB[:],
                    in_=scr[p][:].rearrange("b r -> () (b r)")
                    .broadcast_to([128, (w1 - w0) * 128]))
                return wB

            zt_tiles = {}

            def do_zt(g, wB, p):
                w0, _ = WPIECE[p]
                col0 = GRP_OFF[g]
                gw = GROUP_W[g]
                lo = col0 - w0 * 128
                ztg = zt_pool.tile([128, gw], bf16, tag="ztg", name="ztg",
                                   padded_shape=[128, GROUP_W[0]])
                nc.gpsimd.tensor_tensor(ztg[:],
                                        embt_sb[:, col0:col0 + gw],
                                        wB[:, lo:lo + gw], op=OP.mult)
                zt_tiles[g] = ztg

            def do_gram():
                g_ps = psx_pool.tile([128, 128], f32, tag="gram", name="gram")
                for m in range(M_BLK):
                    nc.tensor.matmul(g_ps[:], lhsT=z0[:, m, :],
                                     rhs=z0[:, m, :],
                                     start=(m == 0), stop=(m == M_BLK - 1))
                g_sb = tmp_pool.tile([128, 128], f32, tag="gsb", name="gsb")
                nc.scalar.copy(g_sb[:], g_ps[:])
                return g_sb

            # ---- squares: tensor windows accumulate into one PSUM bank
            sq_ps = psq_pool.tile([128, 128], f32)
            sqw_done = [0]

            def do_sq_windows(g):
                gw = GROUP_W[g]
                tot = M_BLK * gw
                a, p, d, t = SPLITS[gw]
                cpt = cpts[g]
                for c in range(a + p + d, tot, 128):
                    first = sqw_done[0] == 0
                    sqw_done[0] += 1
                    last = sqw_done[0] == N_SQW
                    nc.tensor.matmul(sq_ps[:], lhsT=cpt[:, c:c + 128],
                                     rhs=cpt[:, c:c + 128],
                                     start=first, stop=last)

            def _chunk_group(i):
                c0 = i * NCH
                for g, (off, gw) in enumerate(zip(GRP_OFF, GROUP_W)):
                    if off <= c0 < off + gw:
                        return g, c0 - off
                raise AssertionError(i)

            def do_v_set(i0, n):
                # V = Z0^T @ CP in fp8 DoubleRow pairs: each matmul
                # contracts 256 rows (two 128-row blocks).  The t-loop is
                # OUTER so the pair weights are reused across the n chunks
                # of the set (each chunk accumulates in its own PSUM bank).
                pss = [psv_pool.tile([128, NCH], f32, tag="psv", name="psv")
                       for _ in range(n)]
                for t in range(M_BLK // 2):
                    for j in range(n):
                        g, koff = _chunk_group(i0 + j)
                        cpv = cpts[g][:].rearrange("q (m w) -> q m w",
                                                   m=M_BLK)
                        nc.tensor.matmul(
                            pss[j][:], lhsT=z0[:, 2 * t:2 * t + 2, :],
                            rhs=cpv[:, 2 * t:2 * t + 2, koff:koff + NCH],
                            start=(t == 0), stop=(t == M_BLK // 2 - 1),
                            perf_mode=mybir.MatmulPerfMode.DoubleRow)
                return pss

            def do_sja(g):
                gw = GROUP_W[g]
                a, _, _, _ = SPLITS[gw]
                cpt = cpts[g]
                pad_a = SPLITS[GROUP_W[0]][0]
                sja = sqa_pool.tile([128, a], f8e3, tag="sja", name="sja",
                                    padded_shape=[128, pad_a])
                nc.scalar.activation(sja[:], cpt[:, 0:a], AF.Square,
                                     accum_out=acc_a[:, g:g + 1])

            def do_sjp(g):
                gw = GROUP_W[g]
                a, p, _, _ = SPLITS[gw]
                if p == 0:
                    return
                cpt = cpts[g]
                pad_p = SPLITS[GROUP_W[0]][1]
                sjp = sqp_pool.tile([128, p], bf16, tag="sjp", name="sjp",
                                    padded_shape=[128, pad_p])
                nc.gpsimd.tensor_tensor(sjp[:], cpt[:, a:a + p],
                                        cpt[:, a:a + p], op=OP.mult)
                nc.gpsimd.tensor_reduce(acc[0:1, P_COL + g:P_COL + g + 1],
                                        sjp[:], axis=mybir.AxisListType.XYZWC,
                                        op=OP.add)

            def do_sjd(g):
                gw = GROUP_W[g]
                a, p, d, _ = SPLITS[gw]
                cpt = cpts[g]
                pad_d = SPLITS[GROUP_W[0]][2]
                sjd = sqd_pool.tile([128, d], f8e3, tag="sjd", name="sjd",
                                    padded_shape=[128, pad_d])
                nc.vector.scalar_tensor_tensor(
                    out=sjd[:], in0=cpt[:, a + p:a + p + d], scalar=1.0,
                    in1=cpt[:, a + p:a + p + d], op0=OP.mult, op1=OP.mult,
                    accum_out=acc[:, D_COL + g:D_COL + g + 1])

            def do_xj(ci, ps):
                g, koff = _chunk_group(ci)
                xj = tmp_pool.tile([128, NCH], bf16, tag="xj", name="xj")
                nc.vector.scalar_tensor_tensor(
                    out=xj[:], in0=ps[:], scalar=1.0,
                    in1=zt_tiles[g][:, koff:koff + NCH],
                    op0=OP.mult, op1=OP.mult,
                    accum_out=acc[:, X_COL + ci:X_COL + ci + 1])

            def do_diag():
                dtmp = tmp_pool.tile([128, 128], bf16, tag="dtmp",
                                     name="dtmp")
                nc.vector.scalar_tensor_tensor(
                    out=dtmp[:], in0=sq_ps[:], scalar=1.0, in1=ident_sb[:],
                    op0=OP.mult, op1=OP.mult,
                    accum_out=acc[:, Q_COL:Q_COL + 1])

            # ---- issue schedule (engine queues are FIFO: order matters)
            def sq_group(g):
                do_sq_windows(g)
                do_sja(g)
                do_sjp(g)
                do_sjd(g)

            do_norm(0)
            do_z0()
            sq_group(0)
            do_norm(1)
            sq_group(1)
            do_norm(2)
            do_wchain(0)
            g_sb = do_gram()
            wB0 = do_pb(0)
            sq_group(2)
            set0 = do_v_set(0, 5)        # chunks 0-4 (groups 0-2)
            do_zt(0, wB0, 0)
            do_xj(0, set0[0])
            do_xj(1, set0[1])
            sq_group(3)
            do_zt(1, wB0, 0)
            do_xj(2, set0[2])
            do_xj(3, set0[3])
            sq_group(4)
            do_wchain(1)
            wB1 = do_pb(1)
            do_zt(2, wB0, 0)
            do_xj(4, set0[4])
            set1 = do_v_set(5, 5)        # chunks 5-9 (groups 2-4)
            do_zt(3, wB0, 0)
            do_xj(5, set1[0])
            do_xj(6, set1[1])
            do_xj(7, set1[2])
            sq_group(5)
            do_zt(4, wB1, 1)
            do_xj(8, set1[3])
            do_xj(9, set1[4])
            sq_group(6)
            set2 = do_v_set(10, 4)       # chunks 10-13 (groups 5-6)
            do_zt(5, wB1, 1)
            do_xj(10, set2[0])
            do_xj(11, set2[1])
            sq_group(7)
            do_zt(6, wB1, 1)
            do_xj(12, set2[2])
            do_xj(13, set2[3])
            sq_group(8)
            set3 = do_v_set(14, 2)       # chunks 14-15 (groups 7-8)
            do_zt(7, wB1, 1)
            do_xj(14, set3[0])
            do_zt(8, wB1, 1)
            do_xj(15, set3[1])
            do_diag()

            nc.scalar.dma_start(out=out_g[:], in_=g_sb[:])
            nc.scalar.dma_start(out=out_acc_a[:], in_=acc_a[:])
            nc.scalar.dma_start(out=out_acc[:], in_=acc[:])

    nc.compile()
    return nc


def _get_module():
    if "nc" not in _cached:
        _cached["nc"] = _build_module()
    return _cached["nc"]


def kernel(emb_i, emb_j, class_pairs, _return_raw=False, _trace=False):
    import ml_dtypes

    emb_i = np.ascontiguousarray(emb_i, dtype=np.float32)
    emb_j = np.ascontiguousarray(emb_j, dtype=np.float32)
    class_pairs = np.ascontiguousarray(class_pairs, dtype=np.float32)
    ident = np.eye(128, dtype=ml_dtypes.bfloat16)

    emb = np.concatenate([emb_i, emb_j], axis=0)          # [8192, 128]
    emb_blocks = emb.reshape(NB, 128, D)
    cp_f8 = class_pairs.astype(ml_dtypes.float8_e4m3)     # host-side cast

    nc = _get_module()
    in_maps = []
    for c in range(N_CORES):
        order = [(M_BLK * c + k) % NB for k in range(NB)]
        rb = emb_blocks[order]                            # [64, 128, 128]
        embs_st = np.ascontiguousarray(
            rb.transpose(1, 0, 2)).astype(ml_dtypes.bfloat16)
        embt_st = np.ascontiguousarray(
            rb.transpose(2, 0, 1).reshape(D, TWO_N)).astype(
                ml_dtypes.float8_e4m3)

        r0 = c * R_LOC
        rows = cp_f8[r0:r0 + R_LOC]                       # [1024, 8192]
        s = r0 % TWO_N
        rot = np.concatenate([rows[:, s:], rows[:, :s]], axis=1)
        rb8 = rot.reshape(M_BLK, 128, TWO_N)              # [8, 128, 8192]
        parts = []
        for gw, off in zip(GROUP_W, GRP_OFF):
            parts.append(rb8[:, :, off:off + gw].transpose(1, 0, 2)
                         .reshape(128, M_BLK * gw))
        cp_st = np.ascontiguousarray(np.concatenate(parts, axis=1))

        in_maps.append({"embs": embs_st, "embt": embt_st, "cpst": cp_st,
                        "ident": ident})

    res = run_bass_kernel_spmd(nc, in_maps, list(range(N_CORES)),
                               trace=_trace)

    G = np.zeros((128, 128), dtype=np.float64)
    sum_cp2 = 0.0
    cross = 0.0
    for c in range(N_CORES):
        G += res.results[c]["out_g"].astype(np.float64)
        out = res.results[c]["out_acc"].astype(np.float64)
        sum_cp2 += res.results[c]["out_acc_a"].astype(np.float64).sum()
        sum_cp2 += out[:, D_COL:X_COL].sum()    # dve partials
        sum_cp2 += out[:, Q_COL].sum()          # tensor diag column
        cross += out[:, X_COL:Q_COL].sum()
    sum_sim2 = float((G * G).sum())
    loss = (sum_sim2 - 2.0 * cross + sum_cp2) / float(TWO_N * TWO_N)
    out = np.asarray(loss, dtype=np.float32)
    if _return_raw:
        return out, res
    return out
